# revision 12
# baseline (speedup 1.0000x reference)
"""Trainium2 Bass kernel for nn_BigramModel (unigram/bigram/trigram interpolated LM).

Strategy (pure data parallel, per sharding hint):
  - Shard text [256, 64] along batch dim across 8 cores -> [256, 8] each.
  - Replicate unigram / bigram_table / tri_rows / tri_map on every core.
  - Per core: 16 tiles of 128 tokens (seq-major per batch column).
    Per tile:
      * indirect-DMA gather the 128 bigram rows (16KB each)
      * compute flat trigram key (prev<<12)+cur on DVE, gather row ids from
        tri_map, then gather tri_rows with bounds_check so misses are skipped
        (miss rate ~99.9% -> near-zero extra HBM traffic)
      * q = tri*mask*(BETA/ALPHA) + bi + (C1/ALPHA)*uni  (fused DVE ops, with
        row-sum accumulated in the same pass)
      * out = Ln(q * (1/Z) + EPS) on the scalar engine, DMA to DRAM.
  All scale factors fold out in the normalization: q = p/ALPHA.
"""

import numpy as np

import concourse.bass as bass
import concourse.bacc as bacc
import concourse.tile as tile
from concourse import mybir
from concourse.bass_utils import run_bass_kernel_spmd

V = 4096
S = 256
B = 64
K = 20000
NCORES = 8
BS = B // NCORES  # 8 batch columns per core
P = 128

ALPHA = 0.4
BETA = 0.3
C1 = 1.0 - ALPHA - BETA  # 0.3
R_UNI = C1 / ALPHA  # 0.75
R_TRI = BETA / ALPHA  # 0.75
EPS = 1e-10
EPS2 = EPS / ALPHA

f32 = mybir.dt.float32
i32 = mybir.dt.int32


def build_nc(n_b: int = BS) -> bass.Bass:
    nc = bacc.Bacc("TRN2", num_devices=NCORES)

    text = nc.dram_tensor("text", [S, n_b], i32, kind="ExternalInput")
    unigram = nc.dram_tensor("unigram", [P, V], f32, kind="ExternalInput")
    bigram = nc.dram_tensor("bigram_table", [V, V], f32, kind="ExternalInput")
    tri_rows = nc.dram_tensor("tri_rows", [K, V], f32, kind="ExternalInput")
    tri_map = nc.dram_tensor("tri_map", [V * V, 1], i32, kind="ExternalInput")
    out = nc.dram_tensor("out", [S, n_b * V], f32, kind="ExternalOutput")

    TRI_BUFS = 3

    with tile.TileContext(nc) as tc:
        with (
            tc.tile_pool(name="const", bufs=1) as const_pool,
            tc.tile_pool(name="bi", bufs=3) as bi_pool,
            tc.tile_pool(name="tri", bufs=TRI_BUFS) as tri_pool,
            tc.tile_pool(name="ot", bufs=3) as out_pool,
            tc.tile_pool(name="small", bufs=4) as small,
        ):
            # unigram comes in pre-replicated [P, V]; scale by C1/ALPHA once
            uni_row = const_pool.tile([P, V], f32, tag="uni_row")
            nc.sync.dma_start(uni_row[:], unigram[:])
            uni_b = const_pool.tile([P, V], f32, tag="uni_b")
            nc.scalar.mul(uni_b[:], uni_row[:], R_UNI)

            eps_b = const_pool.tile([P, 1], f32, tag="eps_b")
            nc.vector.memset(eps_b[:], EPS)

            it = 0
            for b in range(n_b):
                for sblk in range(S // P):
                    s0 = sblk * P

                    cur = small.tile([P, 1], i32, tag="cur")
                    nc.sync.dma_start(cur[:], text[s0 : s0 + P, b : b + 1])
                    prv = small.tile([P, 1], i32, tag="prv")
                    if sblk == 0:
                        nc.sync.dma_start(prv[0:1, :], text[0:1, b : b + 1])
                        nc.sync.dma_start(prv[1:P, :], text[0 : P - 1, b : b + 1])
                    else:
                        nc.sync.dma_start(prv[:], text[s0 - 1 : s0 + P - 1, b : b + 1])

                    # flat trigram key = prev * 4096 + cur
                    fk = small.tile([P, 1], i32, tag="fk")
                    nc.vector.scalar_tensor_tensor(
                        out=fk[:],
                        in0=prv[:],
                        scalar=V,
                        in1=cur[:],
                        op0=mybir.AluOpType.mult,
                        op1=mybir.AluOpType.add,
                    )

                    ridx = small.tile([P, 1], i32, tag="ridx")
                    nc.gpsimd.indirect_dma_start(
                        out=ridx[:],
                        out_offset=None,
                        in_=tri_map[:],
                        in_offset=bass.IndirectOffsetOnAxis(ap=fk[:, :1], axis=0),
                    )
                    if sblk == 0:
                        # seq positions 0 and 1 never take the trigram branch
                        nc.vector.memset(ridx[0:2, :], -1)

                    # miss (-1) -> 65535 which fails bounds_check and is skipped
                    risk = small.tile([P, 1], i32, tag="risk")
                    nc.vector.tensor_scalar(
                        out=risk[:],
                        in0=ridx[:],
                        scalar1=0xFFFF,
                        scalar2=None,
                        op0=mybir.AluOpType.bitwise_and,
                    )

                    # mask in {0, R_TRI} per partition
                    m2a = small.tile([P, 1], f32, tag="m2a")
                    nc.vector.tensor_scalar(
                        out=m2a[:],
                        in0=ridx[:],
                        scalar1=0,
                        scalar2=None,
                        op0=mybir.AluOpType.is_ge,
                    )
                    m2 = small.tile([P, 1], f32, tag="m2")
                    nc.vector.tensor_scalar(
                        out=m2[:],
                        in0=m2a[:],
                        scalar1=R_TRI,
                        scalar2=None,
                        op0=mybir.AluOpType.mult,
                    )

                    bi = bi_pool.tile([P, V], f32, tag="bi")
                    nc.gpsimd.indirect_dma_start(
                        out=bi[:],
                        out_offset=None,
                        in_=bigram[:],
                        in_offset=bass.IndirectOffsetOnAxis(ap=cur[:, :1], axis=0),
                    )

                    tri = tri_pool.tile([P, V], f32, tag="tri")
                    if it < TRI_BUFS:
                        # first touch of each slot: clear so skipped rows stay finite
                        nc.vector.memset(tri[:], 0.0)
                    nc.gpsimd.indirect_dma_start(
                        out=tri[:],
                        out_offset=None,
                        in_=tri_rows[:],
                        in_offset=bass.IndirectOffsetOnAxis(ap=risk[:, :1], axis=0),
                        bounds_check=K - 1,
                        oob_is_err=False,
                    )

                    # q = tri * m2 + bi   (into the bi tile)
                    nc.vector.scalar_tensor_tensor(
                        out=bi[:],
                        in0=tri[:],
                        scalar=m2[:, :1],
                        in1=bi[:],
                        op0=mybir.AluOpType.mult,
                        op1=mybir.AluOpType.add,
                    )

                    # q += uni_b
                    nc.vector.tensor_tensor(
                        out=bi[:],
                        in0=bi[:],
                        in1=uni_b[:],
                        op=mybir.AluOpType.add,
                    )
                    # Z = sum(q); EPS/ALPHA = 2.5e-10 is below f32 resolution
                    # of Z ~ 1.75, so the reference's +EPS is a no-op here
                    z = small.tile([P, 1], f32, tag="z")
                    nc.vector.reduce_sum(
                        out=z[:], in_=bi[:], axis=mybir.AxisListType.X
                    )

                    r = small.tile([P, 1], f32, tag="r")
                    nc.vector.reciprocal(r[:], z[:])

                    ot = out_pool.tile([P, V], f32, tag="ot")
                    nc.scalar.activation(
                        out=ot[:],
                        in_=bi[:],
                        func=mybir.ActivationFunctionType.Ln,
                        bias=eps_b[:, :1],
                        scale=r[:, :1],
                    )

                    nc.sync.dma_start(out[s0 : s0 + P, b * V : (b + 1) * V], ot[:])
                    it += 1

    nc.finalize()
    return nc


def _prep_inputs(text, unigram, bigram_table, tri_rows, tri_map):
    text = np.ascontiguousarray(np.asarray(text, dtype=np.int32))
    uni = np.ascontiguousarray(
        np.broadcast_to(np.asarray(unigram, np.float32).reshape(1, V), (P, V))
    )
    bt = np.ascontiguousarray(np.asarray(bigram_table, np.float32))
    tr = np.ascontiguousarray(np.asarray(tri_rows, np.float32))
    tm = np.ascontiguousarray(np.asarray(tri_map, np.int32).reshape(V * V, 1))
    return text, uni, bt, tr, tm


def make_in_maps(text, uni, bt, tr, tm):
    in_maps = []
    for c in range(NCORES):
        in_maps.append(
            {
                "text": np.ascontiguousarray(text[:, c * BS : (c + 1) * BS]),
                "unigram": uni,
                "bigram_table": bt,
                "tri_rows": tr,
                "tri_map": tm,
            }
        )
    return in_maps


def kernel(text, unigram, bigram_table, tri_rows, tri_map, _trace=False, _trace_kwargs=None):
    text, uni, bt, tr, tm = _prep_inputs(text, unigram, bigram_table, tri_rows, tri_map)
    nc = build_nc(BS)
    in_maps = make_in_maps(text, uni, bt, tr, tm)
    res = run_bass_kernel_spmd(
        nc,
        in_maps,
        core_ids=list(range(NCORES)),
        trace=_trace,
        **(_trace_kwargs or {}),
    )
    outs = [res.results[c]["out"].reshape(S, BS, V) for c in range(NCORES)]
    full = np.concatenate(outs, axis=1)
    if _trace:
        return full, res
    return full


# revision 14
# speedup vs baseline: 12.0642x; 12.0642x over previous
"""Trainium2 Bass kernel for nn_BigramModel (unigram/bigram/trigram interpolated LM).

Strategy (pure data parallel, per sharding hint):
  - Shard text [256, 64] along batch dim across 8 cores -> [256, 8] each.
  - Replicate unigram / bigram_table / tri_rows / tri_map on every core.
  - Per core: 16 tiles of 128 tokens (seq-major per batch column).
    Per tile:
      * indirect-DMA gather the 128 bigram rows (16KB each)
      * compute flat trigram key (prev<<12)+cur on DVE, gather row ids from
        tri_map, then gather tri_rows with bounds_check so misses are skipped
        (miss rate ~99.9% -> near-zero extra HBM traffic)
      * q = tri*mask*(BETA/ALPHA) + bi + (C1/ALPHA)*uni  (fused DVE ops, with
        row-sum accumulated in the same pass)
      * out = Ln(q * (1/Z) + EPS) on the scalar engine, DMA to DRAM.
  All scale factors fold out in the normalization: q = p/ALPHA.
"""

import numpy as np

import concourse.bass as bass
import concourse.bacc as bacc
import concourse.tile as tile
from concourse import mybir
from concourse.bass_utils import run_bass_kernel_spmd

V = 4096
S = 256
B = 64
K = 20000
NCORES = 8
BS = B // NCORES  # 8 batch columns per core
P = 128

ALPHA = 0.4
BETA = 0.3
C1 = 1.0 - ALPHA - BETA  # 0.3
R_UNI = C1 / ALPHA  # 0.75
R_TRI = BETA / ALPHA  # 0.75
EPS = 1e-10
EPS2 = EPS / ALPHA

f32 = mybir.dt.float32
i32 = mybir.dt.int32


def build_nc(n_b: int = BS, repeat: int = 1) -> bass.Bass:
    nc = bacc.Bacc("TRN2", num_devices=NCORES)

    text = nc.dram_tensor("text", [S, n_b], i32, kind="ExternalInput")
    unigram = nc.dram_tensor("unigram", [P, V], f32, kind="ExternalInput")
    bigram = nc.dram_tensor("bigram_table", [V, V], f32, kind="ExternalInput")
    tri_rows = nc.dram_tensor("tri_rows", [K, V], f32, kind="ExternalInput")
    tri_map = nc.dram_tensor("tri_map", [V * V, 1], i32, kind="ExternalInput")
    out = nc.dram_tensor("out", [S, n_b * V], f32, kind="ExternalOutput")

    TRI_BUFS = 3

    with tile.TileContext(nc) as tc:
        with (
            tc.tile_pool(name="const", bufs=1) as const_pool,
            tc.tile_pool(name="bi", bufs=3) as bi_pool,
            tc.tile_pool(name="tri", bufs=TRI_BUFS) as tri_pool,
            tc.tile_pool(name="ot", bufs=3) as out_pool,
            tc.tile_pool(name="small", bufs=4) as small,
        ):
            # unigram comes in pre-replicated [P, V]; scale by C1/ALPHA once
            uni_row = const_pool.tile([P, V], f32, tag="uni_row")
            nc.sync.dma_start(uni_row[:], unigram[:])
            uni_b = const_pool.tile([P, V], f32, tag="uni_b")
            nc.scalar.mul(uni_b[:], uni_row[:], R_UNI)

            eps_b = const_pool.tile([P, 1], f32, tag="eps_b")
            nc.vector.memset(eps_b[:], EPS)

            it = 0
            for _rep in range(repeat):
              for b in range(n_b):
                for sblk in range(S // P):
                    s0 = sblk * P

                    cur = small.tile([P, 1], i32, tag="cur")
                    nc.sync.dma_start(cur[:], text[s0 : s0 + P, b : b + 1])
                    prv = small.tile([P, 1], i32, tag="prv")
                    if sblk == 0:
                        nc.sync.dma_start(prv[0:1, :], text[0:1, b : b + 1])
                        nc.sync.dma_start(prv[1:P, :], text[0 : P - 1, b : b + 1])
                    else:
                        nc.sync.dma_start(prv[:], text[s0 - 1 : s0 + P - 1, b : b + 1])

                    # flat trigram key = prev * 4096 + cur
                    fk = small.tile([P, 1], i32, tag="fk")
                    nc.vector.scalar_tensor_tensor(
                        out=fk[:],
                        in0=prv[:],
                        scalar=V,
                        in1=cur[:],
                        op0=mybir.AluOpType.mult,
                        op1=mybir.AluOpType.add,
                    )

                    ridx = small.tile([P, 1], i32, tag="ridx")
                    nc.gpsimd.indirect_dma_start(
                        out=ridx[:],
                        out_offset=None,
                        in_=tri_map[:],
                        in_offset=bass.IndirectOffsetOnAxis(ap=fk[:, :1], axis=0),
                    )
                    if sblk == 0:
                        # seq positions 0 and 1 never take the trigram branch
                        nc.vector.memset(ridx[0:2, :], -1)

                    # miss (-1) -> 65535 which fails bounds_check and is skipped
                    risk = small.tile([P, 1], i32, tag="risk")
                    nc.vector.tensor_scalar(
                        out=risk[:],
                        in0=ridx[:],
                        scalar1=0xFFFF,
                        scalar2=None,
                        op0=mybir.AluOpType.bitwise_and,
                    )

                    # mask in {0, R_TRI} per partition
                    m2a = small.tile([P, 1], f32, tag="m2a")
                    nc.vector.tensor_scalar(
                        out=m2a[:],
                        in0=ridx[:],
                        scalar1=0,
                        scalar2=None,
                        op0=mybir.AluOpType.is_ge,
                    )
                    m2 = small.tile([P, 1], f32, tag="m2")
                    nc.vector.tensor_scalar(
                        out=m2[:],
                        in0=m2a[:],
                        scalar1=R_TRI,
                        scalar2=None,
                        op0=mybir.AluOpType.mult,
                    )

                    bi = bi_pool.tile([P, V], f32, tag="bi")
                    nc.gpsimd.indirect_dma_start(
                        out=bi[:],
                        out_offset=None,
                        in_=bigram[:],
                        in_offset=bass.IndirectOffsetOnAxis(ap=cur[:, :1], axis=0),
                    )

                    tri = tri_pool.tile([P, V], f32, tag="tri")
                    if it < TRI_BUFS:
                        # first touch of each slot: clear so skipped rows stay finite
                        nc.vector.memset(tri[:], 0.0)
                    nc.gpsimd.indirect_dma_start(
                        out=tri[:],
                        out_offset=None,
                        in_=tri_rows[:],
                        in_offset=bass.IndirectOffsetOnAxis(ap=risk[:, :1], axis=0),
                        bounds_check=K - 1,
                        oob_is_err=False,
                    )

                    # q = tri * m2 + bi   (into the bi tile)
                    nc.vector.scalar_tensor_tensor(
                        out=bi[:],
                        in0=tri[:],
                        scalar=m2[:, :1],
                        in1=bi[:],
                        op0=mybir.AluOpType.mult,
                        op1=mybir.AluOpType.add,
                    )

                    # q += uni_b
                    nc.vector.tensor_tensor(
                        out=bi[:],
                        in0=bi[:],
                        in1=uni_b[:],
                        op=mybir.AluOpType.add,
                    )
                    # Z = sum(q); EPS/ALPHA = 2.5e-10 is below f32 resolution
                    # of Z ~ 1.75, so the reference's +EPS is a no-op here
                    z = small.tile([P, 1], f32, tag="z")
                    nc.vector.reduce_sum(
                        out=z[:], in_=bi[:], axis=mybir.AxisListType.X
                    )

                    r = small.tile([P, 1], f32, tag="r")
                    nc.vector.reciprocal(r[:], z[:])

                    ot = out_pool.tile([P, V], f32, tag="ot")
                    nc.scalar.activation(
                        out=ot[:],
                        in_=bi[:],
                        func=mybir.ActivationFunctionType.Ln,
                        bias=eps_b[:, :1],
                        scale=r[:, :1],
                    )

                    nc.sync.dma_start(out[s0 : s0 + P, b * V : (b + 1) * V], ot[:])
                    it += 1

    nc.finalize()
    return nc


def _prep_inputs(text, unigram, bigram_table, tri_rows, tri_map):
    text = np.ascontiguousarray(np.asarray(text, dtype=np.int32))
    uni = np.ascontiguousarray(
        np.broadcast_to(np.asarray(unigram, np.float32).reshape(1, V), (P, V))
    )
    bt = np.ascontiguousarray(np.asarray(bigram_table, np.float32))
    tr = np.ascontiguousarray(np.asarray(tri_rows, np.float32))
    tm = np.ascontiguousarray(np.asarray(tri_map, np.int32).reshape(V * V, 1))
    return text, uni, bt, tr, tm


def make_in_maps(text, uni, bt, tr, tm):
    in_maps = []
    for c in range(NCORES):
        in_maps.append(
            {
                "text": np.ascontiguousarray(text[:, c * BS : (c + 1) * BS]),
                "unigram": uni,
                "bigram_table": bt,
                "tri_rows": tr,
                "tri_map": tm,
            }
        )
    return in_maps


def kernel(text, unigram, bigram_table, tri_rows, tri_map, _trace=False, _trace_kwargs=None):
    text, uni, bt, tr, tm = _prep_inputs(text, unigram, bigram_table, tri_rows, tri_map)
    nc = build_nc(BS)
    in_maps = make_in_maps(text, uni, bt, tr, tm)
    res = run_bass_kernel_spmd(
        nc,
        in_maps,
        core_ids=list(range(NCORES)),
        trace=_trace,
        **(_trace_kwargs or {}),
    )
    outs = [res.results[c]["out"].reshape(S, BS, V) for c in range(NCORES)]
    full = np.concatenate(outs, axis=1)
    if _trace:
        return full, res
    return full
